# revision 18
# baseline (speedup 1.0000x reference)
"""GroupedTopKRouter (DeepSeek-style MoE routing) on 8 Trainium2 NeuronCores.

Math (per token):
  probs = softmax(x @ W.T)            # [T, E=256]
  per group g (8 groups of 32): top-2 values m1,m2; group_score = m1+m2
  top-4 groups by score; 8 selected experts = top-2 of each selected group
  topk_weights = cand_probs / (sum(cand_probs) + 1e-9)
  aux_loss = E * sum(expert_usage * probs.mean(0)),  usage from top-1 ids

Strategy: data-parallel over tokens, 2048 tokens/core. Host pre-transposes
x into [tile, d_lane, d_chunk, token] layout so the contraction dim lands on
SBUF partitions with fully contiguous DMA; W is passed pre-transposed and
replicated. On device: 32 accumulating matmuls per 128-token tile into PSUM,
fused exp+rowsum on ScalarE, grouped top-2 via segmented reduce_max +
match_replace, iterative masked top-4 over group scores batched across all
16 tiles, final expert ids recovered with one max_index per tile. The aux
loss is finished on host from per-core partial sums (top-1 histogram from
the ids output + per-expert prob column sums computed on PE).
"""

import numpy as np

import concourse.bass as bass
import concourse.mybir as mybir
import concourse.tile as tile
from concourse.bass_utils import run_bass_kernel_spmd

F32 = mybir.dt.float32
I32 = mybir.dt.int32
U32 = mybir.dt.uint32

T = 16384
D = 4096
E = 256
G = 8
EPG = 32
TOP_K = 8
NSEL = 4
NCORES = 8
TSH = T // NCORES          # tokens per core
NT = TSH // 128            # 16 token tiles per core
DC = D // 128              # 32 contraction chunks
BIG = 1.0e30

# matmul input dtype: float32 (exact, 4 cyc/row) or float32r (fast, 1 cyc/row)
MM_DTYPE = F32


def build_nc(mm_dtype=None):
    mm_dtype = mm_dtype or MM_DTYPE
    nc = bass.Bass()

    xt = nc.dram_tensor("xt", [NT, 128, DC, 129], F32, kind="ExternalInput")
    wt = nc.dram_tensor("wt", [D, E], F32, kind="ExternalInput")
    w_out = nc.dram_tensor("w_out", [128, NT * 8], F32, kind="ExternalOutput")
    ids_out = nc.dram_tensor("ids_out", [128, NT * 8], I32, kind="ExternalOutput")
    ps_out = nc.dram_tensor("ps_out", [1, E], F32, kind="ExternalOutput")

    with tile.TileContext(nc) as tc:
        with (
            tc.tile_pool(name="const", bufs=1) as const_pool,
            tc.tile_pool(name="persist", bufs=1) as pp,
            tc.tile_pool(name="xt", bufs=8) as xt_pool,
            tc.tile_pool(name="scratch", bufs=2) as sp,
            tc.tile_pool(name="psum", bufs=4, space="PSUM") as psum_pool,
            tc.tile_pool(name="psum_mp", bufs=1, space="PSUM") as psum_mp_pool,
        ):
            wt_sb = const_pool.tile([128, DC, E], mm_dtype)
            nc.sync.dma_start(
                out=wt_sb[:], in_=wt[:].rearrange("(c p) e -> p c e", p=128)
            )

            u_all = pp.tile([128, NT, G, EPG], F32)
            m1_all = pp.tile([128, NT, G], F32)
            m2_all = pp.tile([128, NT, G], F32)
            z_all = pp.tile([128, NT], F32)
            recip_all = pp.tile([128, NT], F32)
            gsc = pp.tile([128, NT, G], F32)
            cand8 = pp.tile([128, NT, TOP_K], F32)
            candp = pp.tile([128, NT, TOP_K], F32)
            w_sb = pp.tile([128, NT * TOP_K], F32)
            ids_u32 = pp.tile([128, NT, TOP_K], U32)
            ids_i32 = pp.tile([128, NT * TOP_K], I32)
            denom = pp.tile([128, NT], F32)
            rden = pp.tile([128, NT], F32)
            mp_sb = pp.tile([1, E], F32)

            mp_ps = psum_mp_pool.tile([1, E], F32)
            prime_ps = psum_mp_pool.tile([1, 8], F32, tag="prime")

            # the LDWEIGHTS-carrying matmul encoding only supports one sync
            # wait; absorb the wt-DMA wait on a throwaway 1x1 matmul so real
            # matmuls never need two waits
            nc.tensor.matmul(
                prime_ps[:, :1], lhsT=wt_sb[:, 0, :1], rhs=wt_sb[:, 0, :1],
                start=True, stop=True, skip_group_check=True,
            )

            for j in range(NT):
                xt_t = xt_pool.tile([128, DC, 129], mm_dtype)
                # every instruction encoding tolerates only ONE sync wait, so
                # the slot-reuse hazards are absorbed by a chain of one-wait
                # Pool ops: memset A on a pad byte only the DMA ever writes
                # (carries the prior-DMA wait), then memset B on a PE-read
                # byte (carries only the PE-readers wait, the DMA edge being
                # covered by A on the same proc); the DMA then needs just the
                # Pool sem
                nc.gpsimd.memset(xt_t[:1, 0, 128:129], 0.0)
                nc.gpsimd.memset(xt_t[:1, 1, 0:1], 0.0)
                nc.gpsimd.dma_start(out=xt_t[:], in_=xt[j])

                ps_t = psum_pool.tile([128, E], F32)
                # absorb the xt-DMA wait for PE on a throwaway matmul; reads
                # (chunk 1, elem 1) which the memset byte does not cover, so
                # its only dependency is the DMA itself
                nc.tensor.matmul(
                    prime_ps[:, j % 8 : j % 8 + 1],
                    lhsT=xt_t[:, 1, 1:2], rhs=xt_t[:, 1, 1:2],
                    start=True, stop=True, skip_group_check=True,
                )
                for c in range(DC):
                    nc.tensor.matmul(
                        ps_t[:],
                        lhsT=xt_t[:, c, :128],
                        rhs=wt_sb[:, c, :],
                        start=(c == 0),
                        stop=(c == DC - 1),
                    )

                # u = exp(logits); z = row sum of u (fused accumulator)
                u_flat = u_all[:, j].rearrange("p g e -> p (g e)")
                nc.scalar.activation(
                    u_flat,
                    ps_t[:],
                    mybir.ActivationFunctionType.Exp,
                    accum_out=z_all[:, j : j + 1],
                )
                nc.vector.reciprocal(recip_all[:, j : j + 1], z_all[:, j : j + 1])

                # grouped top-2: m1 via segmented max, mask the 8 maxima,
                # segmented max again for m2
                nc.vector.tensor_reduce(
                    m1_all[:, j], u_all[:, j], axis=mybir.AxisListType.X,
                    op=mybir.AluOpType.max,
                )
                u2 = sp.tile([128, G, EPG], F32)
                nc.vector.match_replace(
                    out=u2[:].rearrange("p g e -> p (g e)"),
                    in_to_replace=m1_all[:, j],
                    in_values=u_flat,
                    imm_value=-1.0,
                )
                nc.vector.tensor_reduce(
                    m2_all[:, j], u2[:], axis=mybir.AxisListType.X,
                    op=mybir.AluOpType.max,
                )

            # per-expert prob column sums: mp += recip_row.T @ u  -> [1, E]
            # two wait-absorbing primes (ACT-sem via u_all, DVE-sem via recip)
            nc.tensor.matmul(
                prime_ps[:, :1],
                lhsT=u_all[:, NT - 1, G - 1, EPG - 1 : EPG],
                rhs=u_all[:, NT - 1, G - 1, EPG - 1 : EPG],
                start=True, stop=True, skip_group_check=True,
            )
            nc.tensor.matmul(
                prime_ps[:, 1:2],
                lhsT=recip_all[:, NT - 1 : NT],
                rhs=recip_all[:, NT - 1 : NT],
                start=True, stop=True, skip_group_check=True,
            )
            for j in range(NT):
                nc.tensor.matmul(
                    mp_ps[:],
                    lhsT=recip_all[:, j : j + 1],
                    rhs=u_all[:, j].rearrange("p g e -> p (g e)"),
                    start=(j == 0),
                    stop=(j == NT - 1),
                )

            # ---- batched across all 16 tiles: [128, NT, G] ----
            nc.vector.tensor_tensor(
                out=gsc[:], in0=m1_all[:], in1=m2_all[:], op=mybir.AluOpType.add
            )

            grmax = pp.tile([128, NT], F32)
            for r in range(NSEL):
                nc.vector.tensor_reduce(
                    grmax[:], gsc[:], axis=mybir.AxisListType.X,
                    op=mybir.AluOpType.max,
                )
                eq = sp.tile([128, NT, G], F32, tag="eq")
                nc.vector.tensor_tensor(
                    out=eq[:],
                    in0=gsc[:],
                    in1=grmax[:].to_broadcast([128, NT, G]),
                    op=mybir.AluOpType.is_equal,
                )
                # gather selected group's m1/m2 into output slots 2r, 2r+1
                sel = sp.tile([128, NT, G], F32, tag="sel")
                nc.vector.tensor_tensor(
                    out=sel[:], in0=m1_all[:], in1=eq[:], op=mybir.AluOpType.mult
                )
                nc.vector.tensor_reduce(
                    cand8[:, :, 2 * r : 2 * r + 1], sel[:],
                    axis=mybir.AxisListType.X, op=mybir.AluOpType.max,
                )
                nc.vector.tensor_tensor(
                    out=sel[:], in0=m2_all[:], in1=eq[:], op=mybir.AluOpType.mult
                )
                nc.vector.tensor_reduce(
                    cand8[:, :, 2 * r + 1 : 2 * r + 2],
                    sel[:],
                    axis=mybir.AxisListType.X, op=mybir.AluOpType.max,
                )
                if r < NSEL - 1:
                    nc.vector.scalar_tensor_tensor(
                        out=gsc[:], in0=eq[:], scalar=-BIG, in1=gsc[:],
                        op0=mybir.AluOpType.mult, op1=mybir.AluOpType.add,
                    )

            # normalized candidate probs and weights
            nc.vector.tensor_tensor(
                out=candp[:], in0=cand8[:],
                in1=recip_all[:].to_broadcast([128, NT, TOP_K]),
                op=mybir.AluOpType.mult,
            )
            nc.vector.tensor_reduce(
                denom[:], candp[:], axis=mybir.AxisListType.X,
                op=mybir.AluOpType.add,
            )
            nc.vector.tensor_scalar_add(denom[:], denom[:], 1.0e-9)
            nc.vector.reciprocal(rden[:], denom[:])
            nc.vector.tensor_tensor(
                out=w_sb[:].rearrange("p (t k) -> p t k", k=TOP_K),
                in0=candp[:],
                in1=rden[:].to_broadcast([128, NT, TOP_K]),
                op=mybir.AluOpType.mult,
            )
            nc.sync.dma_start(out=w_out[:], in_=w_sb[:])

            # expert ids: positions of the 8 selected values in each token's u row
            for j in range(NT):
                nc.vector.max_index(
                    out=ids_u32[:, j],
                    in_max=cand8[:, j],
                    in_values=u_all[:, j].rearrange("p g e -> p (g e)"),
                )
            nc.vector.tensor_copy(
                ids_i32[:], ids_u32[:].rearrange("p t k -> p (t k)")
            )
            nc.sync.dma_start(out=ids_out[:], in_=ids_i32[:])

            nc.scalar.copy(mp_sb[:], mp_ps[:])
            nc.sync.dma_start(out=ps_out[:], in_=mp_sb[:])

    # The tail drain waits on every proc, exceeding the encoding's wait slots.
    # Every compute/input-DMA sem is upstream of the three output DMAs, so
    # the drain only needs the output DMA lanes.
    out_names = {"w_out", "ids_out", "ps_out"}
    out_dmas = []
    for f in nc.m.functions:
        for bb in f.blocks:
            for ins in bb.instructions:
                if type(ins).__name__ == "InstDMACopy":
                    o = ins.outs[0]
                    t = getattr(getattr(o, "bass_ap", None), "tensor", None)
                    if getattr(t, "name", None) in out_names:
                        out_dmas.append(ins)
    assert len(out_dmas) == 3
    shared_upd = out_dmas[0].sync_info.on_update[0]
    for ins in out_dmas[1:]:
        si = ins.sync_info
        ins.sync_info = type(si)(on_wait=si.on_wait, on_update=[shared_upd])
    for f in nc.m.functions:
        for bb in f.blocks:
            for ins in bb.instructions:
                si = ins.sync_info
                if si is None or type(ins).__name__ != "InstDrain":
                    continue
                if len(si.on_wait) > 1:
                    base = [
                        w for w in si.on_wait if w.ant_name == shared_upd.ant_name
                    ]
                    assert base, "drain lacks shared output lane wait"
                    w0 = base[0]
                    w0.wait_value = 48
                    ins.sync_info = type(si)(on_wait=[w0], on_update=si.on_update)

    # Hardware instruction encodings accept a single sync wait. The xt-slot
    # absorber memset collects [PE-readers, prior-DMA] on slot reuse, but the
    # prior-DMA edge is transitively implied by the PE edge (the prime matmul
    # read the slot only after that DMA completed), so drop it.
    for f in nc.m.functions:
        for bb in f.blocks:
            for ins in bb.instructions:
                si = ins.sync_info
                if si is None or type(ins).__name__ != "InstMemset":
                    continue
                if len(si.on_wait) == 2:
                    names = sorted(w.ant_name.split("_")[0] for w in si.on_wait)
                    if names[0].startswith("DMASW") and names[1].startswith("PE"):
                        kept = [w for w in si.on_wait if w.ant_name.startswith("PE")]
                        ins.sync_info = type(si)(
                            on_wait=kept, on_update=si.on_update
                        )

    return nc


def _prep_core_inputs(x, W):
    """Shard tokens across cores; pre-transpose so D lands on partitions."""
    wt = np.ascontiguousarray(W.T.astype(np.float32, copy=False))  # [D, E]
    in_maps = []
    for m in range(NCORES):
        xs = x[m * TSH : (m + 1) * TSH]  # [2048, 4096]
        # xt[j, p, c, t] = xs[j*128 + t, c*128 + p]
        xt = np.zeros((NT, 128, DC, 129), dtype=np.float32)
        xt[:, :, :, :128] = xs.reshape(NT, 128, DC, 128).transpose(0, 3, 2, 1)
        in_maps.append({"xt": xt, "wt": wt})
    return in_maps


def _postprocess(results):
    w_parts, id_parts = [], []
    ps_sum = np.zeros((E,), dtype=np.float32)
    for res in results:
        w = res["w_out"].reshape(128, NT, 8).transpose(1, 0, 2).reshape(TSH, 8)
        ids = res["ids_out"].reshape(128, NT, 8).transpose(1, 0, 2).reshape(TSH, 8)
        w_parts.append(w)
        id_parts.append(ids)
        ps_sum += res["ps_out"].reshape(E)
    topk_weights = np.concatenate(w_parts, axis=0).astype(np.float32)
    topk_ids = np.concatenate(id_parts, axis=0).astype(np.int32)

    usage = (
        np.bincount(topk_ids[:, 0], minlength=E).astype(np.float32)
        / np.float32(T)
    )
    mean_prob = ps_sum / np.float32(T)
    aux_loss = np.float32(E) * np.float32(np.dot(usage, mean_prob))
    return topk_weights, topk_ids, np.float32(aux_loss)


_NC_CACHE = {}


def _install_ntff_hook():
    """The container's antenv stub lacks axon_hooks; synthesize it so
    run_bass_kernel_spmd(trace=True) can drive NTFF profiling via the
    injected libaxon_pjrt.so."""
    import sys
    import types

    if "antenv.axon_hooks" in sys.modules:
        return
    try:
        from trn_agent_boot.trn_boot import _ntff_profile_via_ctypes

        hook = _ntff_profile_via_ctypes("/opt/axon/libaxon_pjrt.so")
    except Exception:
        hook = None
    mod = types.ModuleType("antenv.axon_hooks")
    mod.get_axon_ntff_profile_hook = lambda: hook
    mod.set_axon_ntff_profile_hook = lambda h: None
    sys.modules["antenv.axon_hooks"] = mod
    import antenv

    antenv.axon_hooks = mod


def run(inputs, trace=False, **kwargs):
    if trace:
        _install_ntff_hook()
    key = MM_DTYPE
    if key not in _NC_CACHE:
        _NC_CACHE[key] = build_nc()
    nc = _NC_CACHE[key]
    x = np.asarray(inputs["x"], dtype=np.float32)
    W = np.asarray(inputs["W"], dtype=np.float32)
    in_maps = _prep_core_inputs(x, W)
    res = run_bass_kernel_spmd(
        nc, in_maps, core_ids=list(range(NCORES)), trace=trace, **kwargs
    )
    out = _postprocess(res.results)
    return out, res


def kernel(**inputs):
    out, _ = run(inputs, trace=False)
    return out
